# revision 1
# baseline (speedup 1.0000x reference)
"""Self-contained Trainium2 Bass kernel for causal multi-head attention.

Problem: B=2, S=2048, D=1024, H=16 heads (dk=64), fp32, causal + padding mask.
Sharding across 8 NeuronCores: core c -> batch c//4, head-group c%4 (4 heads).
"""

"""Bass/Tile multi-head attention kernel for TRN2, 8-core SPMD.

Sharding: core c -> batch b = c // 4, head group g = c % 4 (4 heads of 16).
Each core computes q/k/v projections for its 4 heads on its batch,
causal+padding-masked attention, and a partial output projection
(its 256 context columns x Wo). Host sums the 4 partials per batch.

Device-side layout (all matmuls at full PE rate via float32r/bf16):
  - qT/kT stored transposed [dk, S]; scores computed transposed S_T[k, q]
    so no transposes are needed anywhere.
  - No max-subtraction in softmax (scores are O(+-10); exp cannot overflow).
  - Softmax denominator: appended pad01 column in V (PV matmul row 64).
  - Padding: V rows and the denominator column zeroed for padded keys, so
    garbage exp values at padded keys multiply zeros everywhere.
  - Causal: additive -8e9 triangle on diagonal 128-blocks (pre-scale);
    sub-diagonal block regions are never computed or read.
  - 1/denominator broadcast to 64 partitions by SBUF->SBUF DMA, multiplied
    into ctx^T on PSUM->SBUF copy; output projection consumes normalized ctx.
  - Projections are emitted interleaved with attention chunks so the PE
    never idles (keeps the HAM clock gate at 2.4 GHz).
Fully-masked rows (all keys up to q padded) produce NaN/garbage on device
and are overwritten on host with the uniform-attention reference value.
"""

import numpy as np
from contextlib import ExitStack

import concourse.bass as bass
import concourse.bacc as bacc
import concourse.tile as tile
import concourse.mybir as mybir
from concourse.bass import ds, ts

F32 = mybir.dt.float32
FR = mybir.dt.float32r
BF = mybir.dt.bfloat16
AF = mybir.ActivationFunctionType

P = 128
S = 2048
D = 1024
HL = 4          # heads per core
DK = 64
KT = D // P     # 8 k-tiles over the model dim
ST = S // P     # 16 seq tiles
NQC = 4         # 512-wide query chunks
NEG = -8.0e9    # pre-scale mask value; *0.125 = -1e9 -> exp underflows to 0
N_CORES = 8
N_HEAD = 16

PT_DT = BF      # probabilities and V dtype (PE streams 1 col/cycle)


def build_program(num_devices=N_CORES):
    nc = bacc.Bacc(
        "TRN2",
        target_bir_lowering=False,
        debug=False,
        enable_asserts=True,
        num_devices=num_devices,
    )
    ins = {
        "xt": nc.dram_tensor("xt", [D, S], FR, kind="ExternalInput").ap(),
        "wq": nc.dram_tensor("wq", [D, 2 * P], FR, kind="ExternalInput").ap(),
        "wk": nc.dram_tensor("wk", [D, 2 * P], FR, kind="ExternalInput").ap(),
        "wv": nc.dram_tensor("wv", [D, 2 * P], FR, kind="ExternalInput").ap(),
        "wo": nc.dram_tensor("wo", [2 * P, D], FR, kind="ExternalInput").ap(),
        "bq": nc.dram_tensor("bq", [P, 2], F32, kind="ExternalInput").ap(),
        "pad01": nc.dram_tensor("pad01", [P, ST], F32, kind="ExternalInput").ap(),
        "tri": nc.dram_tensor("tri", [P, P], F32, kind="ExternalInput").ap(),
    }
    y = nc.dram_tensor("y", [S, D], F32, kind="ExternalOutput").ap()
    ins["rcp_dram"] = nc.dram_tensor("rcp_dram", [NQC * HL, 512], F32).ap()

    with tile.TileContext(nc) as tc:
        _body(tc, y, ins)

    nc.compile()
    return nc


def _body(tc, y, ins):
    nc = tc.nc

    with ExitStack() as ctx:
        const = ctx.enter_context(tc.tile_pool(name="const", bufs=1))
        pt_pool = ctx.enter_context(tc.tile_pool(name="pt", bufs=3))
        rrp = ctx.enter_context(tc.tile_pool(name="rr", bufs=2))
        ysb = ctx.enter_context(tc.tile_pool(name="ysb", bufs=2))
        psA = ctx.enter_context(tc.tile_pool(name="psA", bufs=2, space="PSUM"))
        psB = ctx.enter_context(tc.tile_pool(name="psB", bufs=2, space="PSUM"))
        psY = ctx.enter_context(tc.tile_pool(name="psY", bufs=2, space="PSUM"))

        # ---------------- input DMAs ----------------
        xt_sb = const.tile([P, KT, S], FR)
        wq_sb = const.tile([P, KT, 2 * P], FR)
        wk_sb = const.tile([P, KT, 2 * P], FR)
        wv_sb = const.tile([P, KT, 2 * P], FR)
        xt_r = ins["xt"].rearrange("(k p) s -> k p s", p=P)
        w_rs = {n: ins[n].rearrange("(k p) n -> k p n", p=P) for n in ("wq", "wk", "wv")}
        for k in range(KT):
            nc.sync.dma_start(wq_sb[:, k], w_rs["wq"][k])
            nc.sync.dma_start(wk_sb[:, k], w_rs["wk"][k])
            nc.sync.dma_start(wv_sb[:, k], w_rs["wv"][k])
            # chunk 0 of xt interleaved so projections can start early
            nc.sync.dma_start(xt_sb[:, k, 0:512], xt_r[k][:, 0:512])
        for n in range(1, 3):
            for k in range(KT):
                w_ = 512 if n == 1 else 1024
                nc.sync.dma_start(
                    xt_sb[:, k, ds(n * 512, w_)], xt_r[k][:, ds(n * 512, w_)]
                )

        # wo as [128 c-rows per head-pair... ] -> lhsT is zero-padded ctx, so
        # rhs rows 64-127 for each head must be ZERO (0 x 0, no NaN leakage)
        wo_sb = const.tile([P, HL, D], FR)
        wo_r = ins["wo"].rearrange("(h p) n -> h p n", p=DK)
        for h in range(HL):
            nc.sync.dma_start(wo_sb[0:DK, h], wo_r[h])
            nc.vector.memset(wo_sb[DK:P, h].bitcast(F32), 0.0)

        bq_sb = const.tile([P, 2], F32)
        nc.sync.dma_start(bq_sb[:], ins["bq"])
        pad01_sb = const.tile([P, ST], F32)
        nc.sync.dma_start(pad01_sb[:], ins["pad01"])
        tri_sb = const.tile([P, P], F32)
        nc.sync.dma_start(tri_sb[:], ins["tri"])
        ones_sb = const.tile([1, 512], FR)
        nc.vector.memset(ones_sb[:].bitcast(F32), 1.0)
        ones_f32 = const.tile([1, DK], F32)
        nc.vector.memset(ones_f32[:], 1.0)

        qt_sb = const.tile([P, 2, S], FR)
        kt_sb = const.tile([P, 2, S], FR)
        # per head: 64 value cols + 1 pad01 denominator col; padded so a
        # 128-wide stationary slice starting at h*65 stays in bounds (the
        # extra columns produce junk output rows 65-127, never read)
        VW = HL * (DK + 1) + DK - 1  # 323
        vaug_sb = const.tile([P, ST, VW], PT_DT)
        nc.vector.memset(vaug_sb[:, :, HL * (DK + 1) : VW], 0.0)

        # normalized context, zero-padded to K=128 for the output projection
        ctx_sets = []
        for st in range(2):
            tiles = []
            for h in range(HL):
                t = const.tile([P, 512], FR, name=f"ctxsb{st}_{h}", tag=f"ctxsb{st}_{h}")
                nc.vector.memset(t[DK:P, :].bitcast(F32), 0.0)
                tiles.append(t)
            ctx_sets.append(tiles)

        # PE warmup while the input DMAs stream (HAM un-throttle needs
        # ~3.4us of sustained matmul activity; these are dep-free)
        warm_ps = psY.tile([P, 512], F32, name="warm", tag="yp")
        for i in range(16):
            nc.tensor.matmul(
                warm_ps[:], ones_sb[:, 0:P], ones_sb[:], start=True, stop=True
            )

        # ---------------- projections for one 512-token chunk ----------------
        def proj_chunk(n):
            for tgt, w_sb, bias in ((qt_sb, wq_sb, bq_sb), (kt_sb, wk_sb, None)):
                ps = psA.tile([P, 1024], F32, name=f"ps_p{n}", tag="ps")
                for m in range(2):
                    for k in range(KT):
                        nc.tensor.matmul(
                            ps[:, ts(m, 512)],
                            w_sb[:, k, ts(m, P)],
                            xt_sb[:, k, ds(n * 512, 512)],
                            start=(k == 0),
                            stop=(k == KT - 1),
                        )
                for m in range(2):
                    out_ap = tgt[:, m, ds(n * 512, 512)]
                    if bias is not None:
                        nc.vector.tensor_scalar_add(
                            out_ap, ps[:, ts(m, 512)], bias[:, m : m + 1]
                        )
                    else:
                        nc.vector.tensor_copy(out_ap, ps[:, ts(m, 512)])
            ps = psA.tile([P, 1024], F32, name=f"ps_v{n}", tag="ps")
            for si in range(4):
                s = n * 4 + si
                for k in range(KT):
                    nc.tensor.matmul(
                        ps[:, ts(si, 256)],
                        xt_sb[:, k, ts(s, P)],
                        wv_sb[:, k, :],
                        start=(k == 0),
                        stop=(k == KT - 1),
                    )
            for si in range(4):
                s = n * 4 + si
                for h in range(HL):
                    nc.vector.tensor_scalar_mul(
                        vaug_sb[:, s, ds(h * (DK + 1), DK)],
                        ps[:, ds(si * 256 + h * DK, DK)],
                        pad01_sb[:, s : s + 1],
                    )
                den_ap = vaug_sb[:, s, 0 : HL * (DK + 1)].rearrange(
                    "p (h c) -> p h c", c=DK + 1
                )[:, :, DK : DK + 1]
                nc.vector.tensor_copy(
                    den_ap, pad01_sb[:, s : s + 1].to_broadcast([P, HL, 1])
                )

        # ---------------- attention for one 512-query chunk ----------------
        y_r = y.rearrange("(t p) n -> t p n", p=P)

        def scores_pair(qc, m):
            """QK^T, exp, PV for head pair (2m, 2m+1); copies ctx^T
            (+denominator in row 64) to SBUF so the PSUM banks free fast."""
            nkb = 4 * qc + 4
            pvs = [
                psB.tile([P, 512], F32, name=f"ctx{qc}_{m}_{i}", tag="ctx")
                for i in range(2)
            ]
            for kb in range(nkb):
                dd = kb - 4 * qc
                qoff = max(0, dd) * P
                w = 512 - qoff
                ps = psA.tile([P, 1024], F32, name=f"ps_a{qc}_{m}_{kb}", tag="ps")
                for hh in range(2):
                    r0 = hh * DK
                    nc.tensor.matmul(
                        ps[:, hh * 512 + qoff : (hh + 1) * 512],
                        kt_sb[r0 : r0 + DK, m, ds(kb * P, P)],
                        qt_sb[r0 : r0 + DK, m, ds(qc * 512 + qoff, w)],
                        start=True,
                        stop=True,
                    )
                if dd >= 0:
                    for hh in range(2):
                        diag = ps[:, hh * 512 + qoff : hh * 512 + qoff + P]
                        nc.vector.tensor_add(diag, diag, tri_sb[:])
                pt = pt_pool.tile([P, 1024], PT_DT, name=f"pt{qc}_{m}_{kb}", tag="pt")
                ps3 = ps[:].rearrange("p (h q) -> p h q", h=2)[:, :, qoff:]
                pt3 = pt[:].rearrange("p (h q) -> p h q", h=2)[:, :, qoff:]
                nc.scalar.activation(pt3, ps3, AF.Exp, scale=0.125)
                for hh in range(2):
                    h = 2 * m + hh
                    nc.tensor.matmul(
                        pvs[hh][:, qoff:],
                        vaug_sb[:, kb, ds(h * (DK + 1), P)],
                        pt[:, hh * 512 + qoff : (hh + 1) * 512],
                        start=(kb == 0),
                        stop=(kb == nkb - 1),
                    )
            craws = []
            for hh in range(2):
                h = 2 * m + hh
                craw = rrp.tile(
                    [DK + 1, 512], F32, name=f"craw{qc}_{h}", tag="craw", bufs=5
                )
                nc.vector.tensor_copy(craw[:], pvs[hh][0 : DK + 1, :])
                craws.append(craw)
            return craws

        def norm_pair(qc, m, craws, fast=False):
            """Approx reciprocal (~51 ULP, plenty for softmax denominators) of
            the pair's denominators, broadcast to 64 partitions via a K=1
            matmul, then normalize ctx into the zero-padded SBUF tiles.
            The tiny collect DMAs ride the idle SWDGE queues so they are not
            starved behind bulk input transfers."""
            den2 = rrp.tile([2, 512], F32, name=f"den2_{qc}_{m}", tag="den2", bufs=2)
            for hh in range(2):
                nc.gpsimd.dma_start(den2[hh : hh + 1, :], craws[hh][DK : DK + 1, :])
            rcp2 = rrp.tile([2, 512], F32, name=f"rcp2_{qc}_{m}", tag="rcp2", bufs=2)
            nc.vector.reciprocal_approx_fast(rcp2[:], den2[:])
            base = qc * HL + 2 * m
            if fast:
                # tail path: the PE is idle here, and a K=1 matmul broadcast
                # has much lower latency than the DRAM-bounce DMA chain
                rcp_b = rrp.tile([1, 512], F32, name=f"rcpb{qc}_{m}", tag="rcpb", bufs=2)
                nc.gpsimd.dma_start(rcp_b[:], rcp2[1:2, :])
                for hh in range(2):
                    h = 2 * m + hh
                    rb_ps = psB.tile([DK, 512], F32, name=f"rbp{qc}_{h}", tag="ctx")
                    nc.tensor.matmul(
                        rb_ps[:], ones_f32[:],
                        rcp2[0:1, :] if hh == 0 else rcp_b[:],
                        start=True, stop=True,
                    )
                    nc.vector.tensor_mul(
                        ctx_sets[qc % 2][h][0:DK, :], craws[hh][0:DK, :], rb_ps[:]
                    )
                return
            nc.gpsimd.dma_start(ins["rcp_dram"][base : base + 2, :], rcp2[:])
            for hh in range(2):
                h = 2 * m + hh
                rb = rrp.tile([DK, 512], F32, name=f"rb{qc}_{h}", tag="rb", bufs=3)
                nc.gpsimd.dma_start(
                    rb[:],
                    ins["rcp_dram"][base + hh : base + hh + 1, :].to_broadcast(
                        [DK, 512]
                    ),
                )
                nc.vector.tensor_mul(
                    ctx_sets[qc % 2][h][0:DK, :], craws[hh][0:DK, :], rb[:]
                )

        def outproj(qc, sis=(0, 1, 2, 3)):
            for si in sis:
                s = qc * 4 + si
                yt = ysb.tile([P, 1024], F32, name=f"yt{s}", tag="yt")
                for nch in range(2):
                    yp = psY.tile([P, 512], F32, name=f"yp{s}_{nch}", tag="yp")
                    for h in range(HL):
                        nc.tensor.matmul(
                            yp[:],
                            ctx_sets[qc % 2][h][:, ts(si, P)],
                            wo_sb[:, h, ds(nch * 512, 512)],
                            start=(h == 0),
                            stop=(h == HL - 1),
                        )
                    nc.vector.tensor_copy(yt[:, ts(nch, 512)], yp[:])
                    if nch == 1:
                        nc.sync.dma_start(y_r[s], yt[:])

        # ---------------- interleaved schedule ----------------
        # Emission order IS the per-engine execution order. Late projection
        # chunks are emitted between qc1's attention pairs so their xt-DMA
        # waits are covered by ready attention work, and deferred output
        # projection halves ride AFTER each scores pair so the previous
        # chunk's normalization chain has resolved by the time the PE
        # reaches them.
        proj_chunk(0)
        proj_chunk(1)
        for qc in range(NQC):
            for m in range(2):
                craws = scores_pair(qc, m)
                norm_pair(qc, m, craws, fast=(qc == NQC - 1))
                if qc >= 1:
                    outproj(qc - 1, sis=(0, 1) if m == 0 else (2, 3))
                if qc == 1:
                    proj_chunk(2 + m)
        outproj(NQC - 1)


# ---------------- host side ----------------

def make_in_maps(x, padding_mask, Wq, bq, Wk, Wv, Wo):
    """Build the 8 per-core input dicts from full inputs."""
    x = np.asarray(x, dtype=np.float32)
    pad = np.asarray(padding_mask)
    tri = np.where(
        np.arange(P)[:, None] > np.arange(P)[None, :], np.float32(NEG), np.float32(0)
    ).astype(np.float32)
    in_maps = []
    for c in range(N_CORES):
        b, g = divmod(c, 4)
        R = slice(g * 256, g * 256 + 256)
        pad01 = (pad[b] != 0).astype(np.float32).reshape(ST, P).T.copy()
        in_maps.append(
            {
                "xt": np.ascontiguousarray(x[b].T),
                "wq": np.ascontiguousarray(np.asarray(Wq, np.float32)[R, :].T),
                "wk": np.ascontiguousarray(np.asarray(Wk, np.float32)[R, :].T),
                "wv": np.ascontiguousarray(np.asarray(Wv, np.float32)[R, :].T),
                "wo": np.ascontiguousarray(np.asarray(Wo, np.float32)[:, R].T),
                "bq": np.ascontiguousarray(
                    np.asarray(bq, np.float32)[R].reshape(2, P).T
                ),
                "pad01": pad01,
                "tri": tri,
            }
        )
    return in_maps


def postprocess(partials, x, padding_mask, Wv, bv, Wo, bo):
    """Sum per-core partials, add folded bias, fix fully-masked rows."""
    x = np.asarray(x, np.float32)
    pad = np.asarray(padding_mask)
    Wv = np.asarray(Wv, np.float32)
    bv = np.asarray(bv, np.float32)
    Wo = np.asarray(Wo, np.float32)
    bo = np.asarray(bo, np.float32)
    B = x.shape[0]
    y = np.zeros((B, S, D), dtype=np.float32)
    for c in range(N_CORES):
        y[c // 4] += partials[c]
    y += (Wo @ bv + bo)[None, None, :]
    # fully-masked rows (reference: uniform attention over all keys)
    for b in range(B):
        nz = np.flatnonzero(pad[b] != 0)
        q0 = int(nz[0]) if len(nz) else S
        if q0 > 0:
            ctx_u = x[b].mean(axis=0) @ Wv.T + bv
            y[b, :q0, :] = ctx_u @ Wo.T + bo
    return y


_NC_CACHE = {}


def _get_program():
    if "nc" not in _NC_CACHE:
        _NC_CACHE["nc"] = build_program()
    return _NC_CACHE["nc"]


def kernel(
    x, padding_mask, Wq, bq, Wk, bk, Wv, bv, Wo, bo
):
    from concourse.bass_utils import run_bass_kernel_spmd

    nc = _get_program()
    in_maps = make_in_maps(x, padding_mask, Wq, bq, Wk, Wv, Wo)
    res = run_bass_kernel_spmd(nc, in_maps, core_ids=list(range(N_CORES)))
    partials = [res.results[c]["y"] for c in range(N_CORES)]
    return postprocess(partials, x, padding_mask, Wv, bv, Wo, bo)



# revision 10
# speedup vs baseline: 1.1106x; 1.1106x over previous
"""Self-contained Trainium2 Bass kernel for causal multi-head attention.

Problem: B=2, S=2048, D=1024, H=16 heads (dk=64), fp32, causal + padding mask.
Sharding across 8 NeuronCores: core c -> batch c//4, head-group c%4 (4 heads).
"""

"""Bass/Tile multi-head attention kernel for TRN2, 8-core SPMD.

Sharding: core c -> batch b = c // 4, head group g = c % 4 (4 heads of 16).
Each core computes q/k/v projections for its 4 heads on its batch,
causal+padding-masked attention, and a partial output projection
(its 256 context columns x Wo). Host sums the 4 partials per batch.

Device-side layout (all matmuls at full PE rate via float32r/bf16):
  - qT/kT stored transposed [dk, S]; scores computed transposed S_T[k, q]
    so no transposes are needed anywhere.
  - No max-subtraction in softmax (scores are O(+-10); exp cannot overflow).
  - Softmax denominator: appended pad01 column in V (PV matmul row 64).
  - Padding: V rows and the denominator column zeroed for padded keys, so
    garbage exp values at padded keys multiply zeros everywhere.
  - Causal: additive -8e9 triangle on diagonal 128-blocks (pre-scale);
    sub-diagonal block regions are never computed or read.
  - 1/denominator broadcast to 64 partitions by SBUF->SBUF DMA, multiplied
    into ctx^T on PSUM->SBUF copy; output projection consumes normalized ctx.
  - Projections are emitted interleaved with attention chunks so the PE
    never idles (keeps the HAM clock gate at 2.4 GHz).
Fully-masked rows (all keys up to q padded) produce NaN/garbage on device
and are overwritten on host with the uniform-attention reference value.
"""

import numpy as np
from contextlib import ExitStack

import concourse.bass as bass
import concourse.bacc as bacc
import concourse.tile as tile
import concourse.mybir as mybir
from concourse.bass import ds, ts

F32 = mybir.dt.float32
FR = mybir.dt.float32r
BF = mybir.dt.bfloat16
AF = mybir.ActivationFunctionType

P = 128
S = 2048
D = 1024
HL = 4          # heads per core
DK = 64
KT = D // P     # 8 k-tiles over the model dim
ST = S // P     # 16 seq tiles
NQC = 4         # 512-wide query chunks
NEG = -8.0e9    # pre-scale mask value; *0.125 = -1e9 -> exp underflows to 0
N_CORES = 8
N_HEAD = 16

PT_DT = BF      # probabilities and V dtype (PE streams 1 col/cycle)


def build_program(num_devices=N_CORES):
    nc = bacc.Bacc(
        "TRN2",
        target_bir_lowering=False,
        debug=False,
        enable_asserts=True,
        num_devices=num_devices,
    )
    ins = {
        "xt": nc.dram_tensor("xt", [D, S], BF, kind="ExternalInput").ap(),
        "wq": nc.dram_tensor("wq", [D, 2 * P], BF, kind="ExternalInput").ap(),
        "wk": nc.dram_tensor("wk", [D, 2 * P], BF, kind="ExternalInput").ap(),
        "wv": nc.dram_tensor("wv", [D, 2 * P], BF, kind="ExternalInput").ap(),
        "wo": nc.dram_tensor("wo", [2 * P, D], BF, kind="ExternalInput").ap(),
        "bq": nc.dram_tensor("bq", [P, 2], F32, kind="ExternalInput").ap(),
        "pad01": nc.dram_tensor("pad01", [P, ST], F32, kind="ExternalInput").ap(),
        "tri": nc.dram_tensor("tri", [P, P], F32, kind="ExternalInput").ap(),
    }
    y = nc.dram_tensor("y", [S, D], BF, kind="ExternalOutput").ap()
    ins["rcp_dram"] = nc.dram_tensor("rcp_dram", [NQC * HL, 512], F32).ap()

    with tile.TileContext(nc) as tc:
        _body(tc, y, ins)

    nc.compile()
    return nc


def _body(tc, y, ins):
    nc = tc.nc

    with ExitStack() as ctx:
        const = ctx.enter_context(tc.tile_pool(name="const", bufs=1))
        pt_pool = ctx.enter_context(tc.tile_pool(name="pt", bufs=3))
        rrp = ctx.enter_context(tc.tile_pool(name="rr", bufs=2))
        ysb = ctx.enter_context(tc.tile_pool(name="ysb", bufs=2))
        psA = ctx.enter_context(tc.tile_pool(name="psA", bufs=2, space="PSUM"))
        psB = ctx.enter_context(tc.tile_pool(name="psB", bufs=2, space="PSUM"))
        psY = ctx.enter_context(tc.tile_pool(name="psY", bufs=2, space="PSUM"))

        # ---------------- input DMAs ----------------
        xt_sb = const.tile([P, KT, S], BF)
        wq_sb = const.tile([P, KT, 2 * P], BF)
        wk_sb = const.tile([P, KT, 2 * P], BF)
        wv_sb = const.tile([P, KT, 2 * P], BF)
        xt_r = ins["xt"].rearrange("(k p) s -> k p s", p=P)
        w_rs = {n: ins[n].rearrange("(k p) n -> k p n", p=P) for n in ("wq", "wk", "wv")}
        for k in range(KT):
            nc.sync.dma_start(wq_sb[:, k], w_rs["wq"][k])
            nc.sync.dma_start(wk_sb[:, k], w_rs["wk"][k])
            nc.sync.dma_start(wv_sb[:, k], w_rs["wv"][k])
            # chunk 0 of xt interleaved so projections can start early
            nc.sync.dma_start(xt_sb[:, k, 0:512], xt_r[k][:, 0:512])
        for n in range(1, 3):
            for k in range(KT):
                w_ = 512 if n == 1 else 1024
                nc.sync.dma_start(
                    xt_sb[:, k, ds(n * 512, w_)], xt_r[k][:, ds(n * 512, w_)]
                )

        # wo stacked per head-pair: pair m holds the 128 context rows of
        # heads {2m, 2m+1}; ctx tiles stack the same two heads on the
        # partition axis so each out-proj matmul covers a full K=128.
        wo_sb = const.tile([P, 2, D], BF)
        wo_r = ins["wo"].rearrange("(m p) n -> m p n", p=P)
        for m in range(2):
            nc.sync.dma_start(wo_sb[:, m], wo_r[m])

        bq_sb = const.tile([P, 2], F32)
        nc.sync.dma_start(bq_sb[:], ins["bq"])
        pad01_sb = const.tile([P, ST], F32)
        nc.sync.dma_start(pad01_sb[:], ins["pad01"])
        tri_sb = const.tile([P, P], F32)
        nc.sync.dma_start(tri_sb[:], ins["tri"])
        ones_sb = const.tile([1, 512], BF)
        nc.vector.memset(ones_sb[:], 1.0)
        ones_bf = const.tile([1, DK], BF)
        nc.vector.memset(ones_bf[:], 1.0)

        qt_sb = const.tile([P, 2, S], BF)
        kt_sb = const.tile([P, 2, S], BF)
        # per head: 64 value cols + 1 pad01 denominator col; padded so a
        # 128-wide stationary slice starting at h*65 stays in bounds (the
        # extra columns produce junk output rows 65-127, never read)
        VW = HL * (DK + 1) + DK - 1  # 323
        vaug_sb = const.tile([P, ST, VW], PT_DT)
        nc.vector.memset(vaug_sb[:, :, HL * (DK + 1) : VW], 0.0)

        # normalized context, head-pair m stacked on partitions (head 2m in
        # rows 0:64, head 2m+1 in rows 64:128) for full-K out-proj matmuls
        ctx_sets = []
        for st in range(2):
            tiles = []
            for m in range(2):
                t = const.tile([P, 512], BF, name=f"ctxsb{st}_{m}", tag=f"ctxsb{st}_{m}")
                tiles.append(t)
            ctx_sets.append(tiles)

        # PE warmup while the input DMAs stream (HAM un-throttle needs
        # ~3.4us of sustained matmul activity; these are dep-free)
        warm_ps = psY.tile([P, 512], F32, name="warm", tag="yp")
        for i in range(16):
            nc.tensor.matmul(
                warm_ps[:], ones_sb[:, 0:P], ones_sb[:], start=True, stop=True
            )

        # ---------------- projections for one 512-token chunk ----------------
        def proj_chunk(n):
            for tgt, w_sb, bias in ((qt_sb, wq_sb, bq_sb), (kt_sb, wk_sb, None)):
                ps = psA.tile([P, 1024], F32, name=f"ps_p{n}", tag="ps")
                for m in range(2):
                    for k in range(KT):
                        nc.tensor.matmul(
                            ps[:, ts(m, 512)],
                            w_sb[:, k, ts(m, P)],
                            xt_sb[:, k, ds(n * 512, 512)],
                            start=(k == 0),
                            stop=(k == KT - 1),
                        )
                for m in range(2):
                    out_ap = tgt[:, m, ds(n * 512, 512)]
                    if bias is not None:
                        nc.vector.tensor_scalar_add(
                            out_ap, ps[:, ts(m, 512)], bias[:, m : m + 1]
                        )
                    else:
                        nc.vector.tensor_copy(out_ap, ps[:, ts(m, 512)])
            ps = psA.tile([P, 1024], F32, name=f"ps_v{n}", tag="ps")
            for si in range(4):
                s = n * 4 + si
                for k in range(KT):
                    nc.tensor.matmul(
                        ps[:, ts(si, 256)],
                        xt_sb[:, k, ts(s, P)],
                        wv_sb[:, k, :],
                        start=(k == 0),
                        stop=(k == KT - 1),
                    )
            for si in range(4):
                s = n * 4 + si
                for h in range(HL):
                    nc.vector.tensor_scalar_mul(
                        vaug_sb[:, s, ds(h * (DK + 1), DK)],
                        ps[:, ds(si * 256 + h * DK, DK)],
                        pad01_sb[:, s : s + 1],
                    )
                den_ap = vaug_sb[:, s, 0 : HL * (DK + 1)].rearrange(
                    "p (h c) -> p h c", c=DK + 1
                )[:, :, DK : DK + 1]
                nc.vector.tensor_copy(
                    den_ap, pad01_sb[:, s : s + 1].to_broadcast([P, HL, 1])
                )

        # ---------------- attention for one 512-query chunk ----------------
        y_r = y.rearrange("(t p) n -> t p n", p=P)

        def scores_pair(qc, m):
            """QK^T, exp, PV for head pair (2m, 2m+1); copies ctx^T
            (+denominator in row 64) to SBUF so the PSUM banks free fast."""
            nkb = 4 * qc + 4
            pvs = [
                psB.tile([P, 512], F32, name=f"ctx{qc}_{m}_{i}", tag="ctx")
                for i in range(2)
            ]
            for kb in range(nkb):
                dd = kb - 4 * qc
                qoff = max(0, dd) * P
                w = 512 - qoff
                ps = psA.tile([P, 1024], F32, name=f"ps_a{qc}_{m}_{kb}", tag="ps")
                for hh in range(2):
                    r0 = hh * DK
                    nc.tensor.matmul(
                        ps[:, hh * 512 + qoff : (hh + 1) * 512],
                        kt_sb[r0 : r0 + DK, m, ds(kb * P, P)],
                        qt_sb[r0 : r0 + DK, m, ds(qc * 512 + qoff, w)],
                        start=True,
                        stop=True,
                    )
                if dd >= 0:
                    for hh in range(2):
                        diag = ps[:, hh * 512 + qoff : hh * 512 + qoff + P]
                        nc.vector.tensor_add(diag, diag, tri_sb[:])
                pt = pt_pool.tile([P, 1024], PT_DT, name=f"pt{qc}_{m}_{kb}", tag="pt")
                ps3 = ps[:].rearrange("p (h q) -> p h q", h=2)[:, :, qoff:]
                pt3 = pt[:].rearrange("p (h q) -> p h q", h=2)[:, :, qoff:]
                nc.scalar.activation(pt3, ps3, AF.Exp, scale=0.125)
                for hh in range(2):
                    h = 2 * m + hh
                    nc.tensor.matmul(
                        pvs[hh][:, qoff:],
                        vaug_sb[:, kb, ds(h * (DK + 1), P)],
                        pt[:, hh * 512 + qoff : (hh + 1) * 512],
                        start=(kb == 0),
                        stop=(kb == nkb - 1),
                    )
            craws = []
            for hh in range(2):
                h = 2 * m + hh
                craw = rrp.tile(
                    [DK + 1, 512], F32, name=f"craw{qc}_{h}", tag="craw", bufs=5
                )
                nc.vector.tensor_copy(craw[:], pvs[hh][0 : DK + 1, :])
                craws.append(craw)
            return craws

        def norm_pair(qc, m, craws, fast=False):
            """Approx reciprocal (~51 ULP, plenty for softmax denominators) of
            the pair's denominators, broadcast to 64 partitions via a K=1
            matmul, then normalize ctx into the zero-padded SBUF tiles.
            The tiny collect DMAs ride the idle SWDGE queues so they are not
            starved behind bulk input transfers."""
            den2 = rrp.tile([2, 512], F32, name=f"den2_{qc}_{m}", tag="den2", bufs=2)
            for hh in range(2):
                nc.gpsimd.dma_start(den2[hh : hh + 1, :], craws[hh][DK : DK + 1, :])
            rcp2 = rrp.tile([2, 512], F32, name=f"rcp2_{qc}_{m}", tag="rcp2", bufs=2)
            nc.vector.reciprocal_approx_fast(rcp2[:], den2[:])
            base = qc * HL + 2 * m
            if fast:
                # tail path: the PE is idle here, and a K=1 matmul broadcast
                # has much lower latency than the DRAM-bounce DMA chain
                rcp2b = rrp.tile([2, 512], BF, name=f"rcpb2{qc}_{m}", tag="rcpb2", bufs=2)
                nc.vector.tensor_copy(rcp2b[:], rcp2[:])
                rcp_b = rrp.tile([1, 512], BF, name=f"rcpb{qc}_{m}", tag="rcpb", bufs=2)
                nc.gpsimd.dma_start(rcp_b[:], rcp2b[1:2, :])
                for hh in range(2):
                    rb_ps = psB.tile([DK, 512], F32, name=f"rbp{qc}_{m}_{hh}", tag="ctx")
                    nc.tensor.matmul(
                        rb_ps[:], ones_bf[:],
                        rcp2b[0:1, :] if hh == 0 else rcp_b[:],
                        start=True, stop=True,
                    )
                    nc.vector.tensor_mul(
                        ctx_sets[qc % 2][m][hh * DK : (hh + 1) * DK, :],
                        craws[hh][0:DK, :], rb_ps[:],
                    )
                return
            nc.gpsimd.dma_start(ins["rcp_dram"][base : base + 2, :], rcp2[:])
            for hh in range(2):
                h = 2 * m + hh
                rb = rrp.tile([DK, 512], F32, name=f"rb{qc}_{h}", tag="rb", bufs=3)
                nc.gpsimd.dma_start(
                    rb[:],
                    ins["rcp_dram"][base + hh : base + hh + 1, :].to_broadcast(
                        [DK, 512]
                    ),
                )
                nc.vector.tensor_mul(
                    ctx_sets[qc % 2][m][hh * DK : (hh + 1) * DK, :],
                    craws[hh][0:DK, :], rb[:],
                )

        def outproj(qc, sis=(0, 1, 2, 3)):
            for si in sis:
                s = qc * 4 + si
                yt = ysb.tile([P, 1024], BF, name=f"yt{s}", tag="yt")
                for nch in range(2):
                    yp = psY.tile([P, 512], F32, name=f"yp{s}_{nch}", tag="yp")
                    for m in range(2):
                        nc.tensor.matmul(
                            yp[:],
                            ctx_sets[qc % 2][m][:, ts(si, P)],
                            wo_sb[:, m, ds(nch * 512, 512)],
                            start=(m == 0),
                            stop=(m == 1),
                        )
                    nc.vector.tensor_copy(yt[:, ts(nch, 512)], yp[:])
                    if nch == 1:
                        nc.sync.dma_start(y_r[s], yt[:])

        # ---------------- interleaved schedule ----------------
        # Emission order IS the per-engine execution order. Late projection
        # chunks are emitted between qc1's attention pairs so their xt-DMA
        # waits are covered by ready attention work, and deferred output
        # projection halves ride AFTER each scores pair so the previous
        # chunk's normalization chain has resolved by the time the PE
        # reaches them.
        proj_chunk(0)
        proj_chunk(1)
        for qc in range(NQC):
            for m in range(2):
                craws = scores_pair(qc, m)
                norm_pair(qc, m, craws, fast=(qc == NQC - 1))
                if qc >= 1:
                    outproj(qc - 1, sis=(0, 1) if m == 0 else (2, 3))
                if qc == 1:
                    proj_chunk(2 + m)
        outproj(NQC - 1)


# ---------------- host side ----------------

def make_in_maps(x, padding_mask, Wq, bq, Wk, Wv, Wo):
    """Build the 8 per-core input dicts from full inputs."""
    import ml_dtypes

    BF_NP = ml_dtypes.bfloat16
    x = np.asarray(x, dtype=np.float32)
    pad = np.asarray(padding_mask)
    tri = np.where(
        np.arange(P)[:, None] > np.arange(P)[None, :], np.float32(NEG), np.float32(0)
    ).astype(np.float32)
    in_maps = []
    for c in range(N_CORES):
        b, g = divmod(c, 4)
        R = slice(g * 256, g * 256 + 256)
        pad01 = (pad[b] != 0).astype(np.float32).reshape(ST, P).T.copy()
        in_maps.append(
            {
                "xt": np.ascontiguousarray(x[b].T).astype(BF_NP),
                "wq": np.ascontiguousarray(np.asarray(Wq, np.float32)[R, :].T).astype(BF_NP),
                "wk": np.ascontiguousarray(np.asarray(Wk, np.float32)[R, :].T).astype(BF_NP),
                "wv": np.ascontiguousarray(np.asarray(Wv, np.float32)[R, :].T).astype(BF_NP),
                "wo": np.ascontiguousarray(np.asarray(Wo, np.float32)[:, R].T).astype(BF_NP),
                "bq": np.ascontiguousarray(
                    np.asarray(bq, np.float32)[R].reshape(2, P).T
                ),
                "pad01": pad01,
                "tri": tri,
            }
        )
    return in_maps


def postprocess(partials, x, padding_mask, Wv, bv, Wo, bo):
    """Sum per-core partials, add folded bias, fix fully-masked rows."""
    x = np.asarray(x, np.float32)
    pad = np.asarray(padding_mask)
    Wv = np.asarray(Wv, np.float32)
    bv = np.asarray(bv, np.float32)
    Wo = np.asarray(Wo, np.float32)
    bo = np.asarray(bo, np.float32)
    B = x.shape[0]
    y = np.zeros((B, S, D), dtype=np.float32)
    for c in range(N_CORES):
        y[c // 4] += np.asarray(partials[c]).astype(np.float32)
    y += (Wo @ bv + bo)[None, None, :]
    # fully-masked rows (reference: uniform attention over all keys)
    for b in range(B):
        nz = np.flatnonzero(pad[b] != 0)
        q0 = int(nz[0]) if len(nz) else S
        if q0 > 0:
            ctx_u = x[b].mean(axis=0) @ Wv.T + bv
            y[b, :q0, :] = ctx_u @ Wo.T + bo
    return y


_NC_CACHE = {}


def _get_program():
    if "nc" not in _NC_CACHE:
        _NC_CACHE["nc"] = build_program()
    return _NC_CACHE["nc"]


def kernel(
    x, padding_mask, Wq, bq, Wk, bk, Wv, bv, Wo, bo
):
    from concourse.bass_utils import run_bass_kernel_spmd

    nc = _get_program()
    in_maps = make_in_maps(x, padding_mask, Wq, bq, Wk, Wv, Wo)
    res = run_bass_kernel_spmd(nc, in_maps, core_ids=list(range(N_CORES)))
    partials = [res.results[c]["y"] for c in range(N_CORES)]
    return postprocess(partials, x, padding_mask, Wv, bv, Wo, bo)

